# revision 42
# baseline (speedup 1.0000x reference)
"""MinusAttention kernel for Trainium2 (8 NeuronCores, Bass/Tile).

Math: score[i,j] = (w.q_i - w.k_j + b) / sqrt(E) with causal mask.
Within a softmax row i the w.q_i and b terms cancel, so

    weights[i,j] = g_j / sum_{j'<=i} g_j',   g_j = exp(-w.k_j / sqrt(E))
    out[i,:]     = (sum_{j<=i} g_j V[j,:]) / (sum_{j<=i} g_j)

i.e. a causal cumulative weighted average of V -- O(S*E) per (b,h) --
and the output does not depend on queries at all.

fp16 end-to-end (PE 1 cycle/col vs 4 for fp32, half the HBM bytes)
with k-major layouts everywhere so every engine/matmul access pattern
is contiguous (measured: transposed ACT writes cost 3.5x, permuted
matmul rhs costs 2x on this HW).  Ragged [7,7,2]-block PSUM chunks (3
per pair, each within a 2KB bank), one merged block-sum scatter per
pair, carry mask multiply in the DVE packed-16bit 2x mode, two-stage
reduce (packed TT + reduce), consts DMA'd from host.  kt+scatter ride
the SP HW ring, vg the ACT HW ring, output stores the gpsimd queue;
the first pair's inputs lead both queues since the first reduce gates
the pipeline.  Final normalize on GPSIMD for the first two pairs (DVE
is busy with front phases then), on DVE for the last two (tail).

Per pair (b,h), s = 128*k + p (p partition, k block 0..15):
  kt  [128,16,64] f16  (host-prescaled by -w/sqrt(E))
  vg  [128,16,65] f16  (col e=64 is ones)
  sk  = reduce_add_e(kt)            DVE 2-stage -> [128,16] f16
  g   = exp(sk)                     ACT
  wg  = vg * g_bcast                DVE
  ps_c = triT @ wg[:,k0:k1,:]       PE f16 (within-block prefix sums)
  c32[:,k0:k1,:] = ps_c[96:128]     ACT (PSUM reads: 32-aligned base)
  bsT = scatter c32 row 31          1 DMA (block sums -> partitions)
  rm  = maskKED * bsT_bcast         DVE 2x (carry terms, k' < k)
  ps_c += ones16 @ rm[:,k0:k1,:]    PE (adds inter-block carries)
  cw[:,k0:k1,:] = ps_c              ACT drains
  r   = 1/cw[:,:,64]                DVE
  ot  = cw[:,:,0:64] * r_bcast      GPSIMD / DVE
  out DMA (f16; host upcasts)
"""

import numpy as np

B, L, S, H, E = 4, 2048, 2048, 8, 64
NCORES = 8
PAIRS = (B * H) // NCORES  # 4 (b,h) pairs per core
NBLK = S // 128  # 16
# PSUM V-chunks: [8,8] blocks x 64 cols -> 2 tiles/pair, each exactly one
# 2KB bank; the denominator column lives in a single shared bank for all
# pairs (its matmul rhs is g itself)
CHUNKS = [(0, 8), (8, 16)]
GROUP = 2
SCALE = np.float32(1.0 / np.sqrt(np.float32(E)))

TRACE = False
LAST_RESULTS = None

_compiled = None


def _build():
    from concourse import bacc
    import concourse.mybir as mybir
    import concourse.tile as tile

    f16 = mybir.dt.float16
    f32 = mybir.dt.float32
    nc = bacc.Bacc("TRN2", target_bir_lowering=False, debug=False)

    ktw = nc.dram_tensor("ktw", [PAIRS, 128, NBLK, E], f16, kind="ExternalInput")
    vg = nc.dram_tensor("vg", [PAIRS, 128, NBLK, E], f16, kind="ExternalInput")
    tri_c = nc.dram_tensor("tri_c", [128, 128], f16, kind="ExternalInput")
    ones_c = nc.dram_tensor("ones_c", [16, 128], f16, kind="ExternalInput")
    # maskKED[k', k, e] = 1 iff k' < k (bcast along e incl. the D col)
    mke_c = nc.dram_tensor("mke_c", [16, NBLK, E + 1], f16, kind="ExternalInput")
    out = nc.dram_tensor("out", [PAIRS, 128, NBLK, E], f16, kind="ExternalOutput")

    with tile.TileContext(nc) as tc:
        with (
            nc.allow_low_precision(reason="fp16 kernel; harness gate is 2e-2"),
            tc.tile_pool(name="const", bufs=1) as cpool,
            tc.tile_pool(name="ktp", bufs=PAIRS) as ktp,
            tc.tile_pool(name="vgp", bufs=PAIRS) as vgp,
            tc.tile_pool(name="skp", bufs=2 * GROUP) as skp,
            tc.tile_pool(name="gp", bufs=2 * GROUP) as gp,
            tc.tile_pool(name="wgp", bufs=PAIRS) as wgp,
            tc.tile_pool(name="c32p", bufs=2 * GROUP) as c32p,
            tc.tile_pool(name="bsp", bufs=2 * GROUP) as bsp,
            tc.tile_pool(name="rmp", bufs=2 * GROUP) as rmp,
            tc.tile_pool(name="rp", bufs=2 * GROUP) as rp,
            tc.tile_pool(name="cwp", bufs=GROUP + 1) as cwp,
            tc.tile_pool(name="otp", bufs=GROUP + 1) as otp,
            tc.tile_pool(name="ps", bufs=6, space="PSUM") as psp,
            tc.tile_pool(name="psd", bufs=2, space="PSUM") as psdp,
        ):
            allp = list(range(PAIRS))
            kts, vgts = {}, {}
            # first pair's inputs lead both queues: the first reduce gates
            # the whole pipeline, so nothing may be enqueued ahead of kt0
            for p in allp:
                kt = ktp.tile([128, NBLK, E], f16, tag="kt")
                vgt = vgp.tile([128, NBLK, E], f16, tag="vg")
                nc.sync.dma_start(out=kt[:], in_=ktw[p])
                nc.scalar.dma_start(out=vgt[:], in_=vg[p])
                kts[p], vgts[p] = kt, vgt

            tri = cpool.tile([128, 128], f16)
            nc.sync.dma_start(out=tri[:], in_=tri_c[:])
            ones16 = cpool.tile([16, 128], f16)
            nc.sync.dma_start(out=ones16[:], in_=ones_c[:])
            maskKED = cpool.tile([16, NBLK, E + 1], f16)
            nc.sync.dma_start(out=maskKED[:], in_=mke_c[:])

            # front phases, paced pair-by-pair on DVE/ACT; the order edge
            # keeps the scheduler from running later pairs' reduces (gated
            # on late kt DMAs) ahead of this pair's wg in the DVE stream
            from concourse.tile_rust import add_dep_helper
            wgs, gs = {}, {}
            prev_wg = None
            for p in allp:
                # two-stage reduce: DVE adds the e-halves in the packed-16bit
                # 2x mode, the otherwise-idle GPSIMD engine finishes the sum
                t1 = skp.tile([128, NBLK, E // 2], f16, tag="t1")
                red = nc.vector.tensor_tensor(
                    out=t1[:], in0=kts[p][:, :, 0:E // 2],
                    in1=kts[p][:, :, E // 2:E], op=mybir.AluOpType.add,
                )
                sk = skp.tile([128, NBLK], f16, tag="sk")
                nc.vector.tensor_reduce(
                    sk[:], t1[:], mybir.AxisListType.X, mybir.AluOpType.add
                )
                if prev_wg is not None:
                    add_dep_helper(red.ins, prev_wg.ins, sync=False,
                                   reason="reduce after prev pair wg")
                g = gp.tile([128, NBLK], f16, tag="g")
                nc.scalar.activation(
                    g[:], sk[:], mybir.ActivationFunctionType.Exp
                )
                wg = wgp.tile([128, NBLK, E], f16, tag="wg")
                prev_wg = nc.vector.tensor_tensor(
                    out=wg[:], in0=vgts[p][:],
                    in1=g[:].to_broadcast([128, NBLK, E]),
                    op=mybir.AluOpType.mult,
                )
                wgs[p] = wg
                gs[p] = g

            # PSUM stages: 2-pair groups, but with 6 V-bank bufs the next
            # group's matmuls start as soon as one earlier pair drains
            for grp in range(PAIRS // GROUP):
                pairs = list(range(grp * GROUP, (grp + 1) * GROUP))

                pss = {}
                psds = {}
                for p in pairs:
                    # D prefix first: needs only g, runs while wg is built
                    psd = psdp.tile([128, NBLK], f32, tag="psd")
                    psds[p] = psd
                    nc.tensor.matmul(
                        psd[:], lhsT=tri[:], rhs=gs[p][:],
                        start=True, stop=False, skip_group_check=True,
                    )
                    for ci, (k0, k1) in enumerate(CHUNKS):
                        ps = psp.tile([128, k1 - k0, E], f32, tag="ps")
                        nc.tensor.matmul(
                            ps[:], lhsT=tri[:],
                            rhs=wgs[p][:, k0:k1, :],
                            start=True, stop=False, skip_group_check=True,
                        )
                        pss[(p, ci)] = ps

                bsTs = {}
                for p in pairs:
                    # block sums live in row 127; PSUM reads need 32-aligned
                    # partition base: copy rows 96:128, then scatter row 31
                    c32 = c32p.tile([32, NBLK, E + 1], f16, tag="c32")
                    for ci, (k0, k1) in enumerate(CHUNKS):
                        nc.scalar.copy(
                            c32[:, k0:k1, 0:E], pss[(p, ci)][96:128, :, :])
                    nc.scalar.copy(
                        c32[:, :, E:E + 1].rearrange("p k o -> p (k o)"),
                        psds[p][96:128, :])
                    bsT = bsp.tile([NBLK, 1, E + 1], f16, tag="bs")
                    nc.sync.dma_start(out=bsT[:], in_=c32[31:32, :, :],
                                      single_packet=True)
                    bsTs[p] = bsT

                rms = {}
                for p in pairs:
                    rmV = rmp.tile([NBLK, NBLK, E], f16, tag="rmv")
                    nc.vector.tensor_tensor(
                        out=rmV[:], in0=maskKED[:, :, 0:E],
                        in1=bsTs[p][:, :, 0:E].broadcast_to([NBLK, NBLK, E]),
                        op=mybir.AluOpType.mult,
                    )
                    rmD = rmp.tile([NBLK, NBLK], f16, tag="rmd")
                    nc.vector.tensor_tensor(
                        out=rmD[:],
                        in0=maskKED[:, :, E:E + 1].rearrange("a k o -> a (k o)"),
                        in1=bsTs[p][:, :, E:E + 1].rearrange(
                            "a k o -> a (k o)").broadcast_to([NBLK, NBLK]),
                        op=mybir.AluOpType.mult,
                    )
                    rms[p] = (rmV, rmD)

                for p in pairs:
                    for ci, (k0, k1) in enumerate(CHUNKS):
                        nc.tensor.matmul(
                            pss[(p, ci)][:], lhsT=ones16[:],
                            rhs=rms[p][0][:, k0:k1, :],
                            start=False, stop=True, skip_group_check=True,
                        )
                    nc.tensor.matmul(
                        psds[p][:], lhsT=ones16[:], rhs=rms[p][1][:],
                        start=False, stop=True, skip_group_check=True,
                    )

                for p in pairs:
                    r = rp.tile([128, NBLK], f16, tag="r")
                    nc.vector.reciprocal(r[:], psds[p][:])
                    cw = cwp.tile([128, NBLK, E], f16, tag="cw")
                    nc.scalar.copy(cw[:, 0:8, :], pss[(p, 0)][:])
                    nc.scalar.copy(cw[:, 8:16, :], pss[(p, 1)][:])
                    ot = otp.tile([128, NBLK, E], f16, tag="ot")
                    # final normalize: Pool, except the last pair (tail) on
                    # the by-then-idle DVE
                    eng = nc.vector if p == PAIRS - 1 else nc.gpsimd
                    eng.tensor_tensor(
                        out=ot[:], in0=cw[:],
                        in1=r[:].to_broadcast([128, NBLK, E]),
                        op=mybir.AluOpType.mult,
                    )
                    nc.gpsimd.dma_start(out=out[p], in_=ot[:])

    nc.compile()
    return nc


def _get_compiled():
    global _compiled
    if _compiled is None:
        _compiled = _build()
    return _compiled


def _consts():
    f16 = np.float16
    tri = np.triu(np.ones((128, 128), np.float32)).astype(f16)  # tri[c,p]=1 iff c<=p
    ones16 = np.ones((16, 128), f16)
    mk = (np.arange(NBLK)[:, None] < np.arange(NBLK)[None, :]).astype(np.float32)
    mke = np.broadcast_to(mk[:, :, None], (16, NBLK, E + 1)).astype(f16)
    return {
        "tri_c": tri,
        "ones_c": ones16,
        "mke_c": np.ascontiguousarray(mke),
    }


def prep_inputs(keys: np.ndarray, values: np.ndarray, w_score: np.ndarray):
    """Host-side reshard: returns in_maps (list of 8 dicts)."""
    keys = np.asarray(keys, dtype=np.float32)
    values = np.asarray(values, dtype=np.float32)
    w = np.asarray(w_score, dtype=np.float32)

    # [B,S,H,E] -> [B,H,S,E] -> [B*H, NBLK, 128, E] -> [B*H, 128, NBLK, E]
    kt = keys.transpose(0, 2, 1, 3).reshape(B * H, NBLK, 128, E)
    kt = (kt * (-SCALE * w)).transpose(0, 2, 1, 3).astype(np.float16)

    vgf = values.transpose(0, 2, 1, 3).reshape(B * H, NBLK, 128, E)
    vgf = vgf.transpose(0, 2, 1, 3).astype(np.float16)  # [B*H, 128, NBLK, E]

    consts = _consts()
    in_maps = []
    for c in range(NCORES):
        sl = slice(PAIRS * c, PAIRS * (c + 1))
        m = {
            "ktw": np.ascontiguousarray(kt[sl]),
            "vg": np.ascontiguousarray(vgf[sl]),
        }
        m.update(consts)
        in_maps.append(m)
    return in_maps


def assemble_output(results) -> np.ndarray:
    # results[c]["out"]: [PAIRS, 128, NBLK, E]; s = 128*k + partition
    arr = np.stack([np.asarray(r["out"]) for r in results])
    arr = arr.reshape(B * H, 128, NBLK, E).astype(np.float32)
    arr = arr.transpose(0, 2, 1, 3).reshape(B, H, L, E).transpose(0, 2, 1, 3)
    return np.ascontiguousarray(arr)


def kernel(queries=None, keys=None, values=None, w_score=None, b_score=None, attn_mask=None, **_):
    global LAST_RESULTS
    from concourse.bass_utils import run_bass_kernel_spmd

    nc = _get_compiled()
    in_maps = prep_inputs(keys, values, w_score)
    res = run_bass_kernel_spmd(nc, in_maps, core_ids=list(range(NCORES)), trace=TRACE)
    LAST_RESULTS = res
    return assemble_output(res.results)


# revision 43
# speedup vs baseline: 1.0537x; 1.0537x over previous
"""MinusAttention kernel for Trainium2 (8 NeuronCores, Bass/Tile).

Math: score[i,j] = (w.q_i - w.k_j + b) / sqrt(E) with causal mask.
Within a softmax row i the w.q_i and b terms cancel, so

    weights[i,j] = g_j / sum_{j'<=i} g_j',   g_j = exp(-w.k_j / sqrt(E))
    out[i,:]     = (sum_{j<=i} g_j V[j,:]) / (sum_{j<=i} g_j)

i.e. a causal cumulative weighted average of V -- O(S*E) per (b,h) --
and the output does not depend on queries at all.

fp16 end-to-end (PE 1 cycle/col vs 4 for fp32, half the HBM bytes)
with k-major layouts everywhere so every engine/matmul access pattern
is contiguous (measured: transposed ACT writes cost 3.5x, permuted
matmul rhs costs 2x on this HW).  Ragged [7,7,2]-block PSUM chunks (3
per pair, each within a 2KB bank), one merged block-sum scatter per
pair, carry mask multiply in the DVE packed-16bit 2x mode, two-stage
reduce (packed TT + reduce), consts DMA'd from host.  kt+scatter ride
the SP HW ring, vg the ACT HW ring, output stores the gpsimd queue;
the first pair's inputs lead both queues since the first reduce gates
the pipeline.  Final normalize on GPSIMD for the first two pairs (DVE
is busy with front phases then), on DVE for the last two (tail).

Per pair (b,h), s = 128*k + p (p partition, k block 0..15):
  kt  [128,16,64] f16  (host-prescaled by -w/sqrt(E))
  vg  [128,16,65] f16  (col e=64 is ones)
  sk  = reduce_add_e(kt)            DVE 2-stage -> [128,16] f16
  g   = exp(sk)                     ACT
  wg  = vg * g_bcast                DVE
  ps_c = triT @ wg[:,k0:k1,:]       PE f16 (within-block prefix sums)
  c32[:,k0:k1,:] = ps_c[96:128]     ACT (PSUM reads: 32-aligned base)
  bsT = scatter c32 row 31          1 DMA (block sums -> partitions)
  rm  = maskKED * bsT_bcast         DVE 2x (carry terms, k' < k)
  ps_c += ones16 @ rm[:,k0:k1,:]    PE (adds inter-block carries)
  cw[:,k0:k1,:] = ps_c              ACT drains
  r   = 1/cw[:,:,64]                DVE
  ot  = cw[:,:,0:64] * r_bcast      GPSIMD / DVE
  out DMA (f16; host upcasts)
"""

import numpy as np

B, L, S, H, E = 4, 2048, 2048, 8, 64
NCORES = 8
PAIRS = (B * H) // NCORES  # 4 (b,h) pairs per core
NBLK = S // 128  # 16
# PSUM V-chunks: [8,8] blocks x 64 cols -> 2 tiles/pair, each exactly one
# 2KB bank; the denominator column lives in a single shared bank for all
# pairs (its matmul rhs is g itself)
CHUNKS = [(0, 8), (8, 16)]
GROUP = 2
SCALE = np.float32(1.0 / np.sqrt(np.float32(E)))

TRACE = False
LAST_RESULTS = None

_compiled = None


def _build():
    from concourse import bacc
    import concourse.mybir as mybir
    import concourse.tile as tile

    f16 = mybir.dt.float16
    f32 = mybir.dt.float32
    nc = bacc.Bacc("TRN2", target_bir_lowering=False, debug=False)

    ktw = nc.dram_tensor("ktw", [PAIRS, 128, NBLK, E], f16, kind="ExternalInput")
    vg = nc.dram_tensor("vg", [PAIRS, 128, NBLK, E], f16, kind="ExternalInput")
    tri_c = nc.dram_tensor("tri_c", [128, 128], f16, kind="ExternalInput")
    ones_c = nc.dram_tensor("ones_c", [16, 128], f16, kind="ExternalInput")
    # maskKED[k', k, e] = 1 iff k' < k (bcast along e incl. the D col)
    mke_c = nc.dram_tensor("mke_c", [16, NBLK, E + 1], f16, kind="ExternalInput")
    out = nc.dram_tensor("out", [PAIRS, 128, NBLK, E], f16, kind="ExternalOutput")

    with tile.TileContext(nc) as tc:
        with (
            nc.allow_low_precision(reason="fp16 kernel; harness gate is 2e-2"),
            tc.tile_pool(name="const", bufs=1) as cpool,
            tc.tile_pool(name="ktp", bufs=PAIRS) as ktp,
            tc.tile_pool(name="vgp", bufs=PAIRS) as vgp,
            tc.tile_pool(name="skp", bufs=2 * GROUP) as skp,
            tc.tile_pool(name="gp", bufs=2 * GROUP) as gp,
            tc.tile_pool(name="wgp", bufs=PAIRS) as wgp,
            tc.tile_pool(name="c32p", bufs=2 * GROUP) as c32p,
            tc.tile_pool(name="bsp", bufs=2 * GROUP) as bsp,
            tc.tile_pool(name="rmp", bufs=2 * GROUP) as rmp,
            tc.tile_pool(name="rp", bufs=2 * GROUP) as rp,
            tc.tile_pool(name="cwp", bufs=GROUP + 1) as cwp,
            tc.tile_pool(name="otp", bufs=GROUP + 1) as otp,
            tc.tile_pool(name="ps", bufs=6, space="PSUM") as psp,
            tc.tile_pool(name="psd", bufs=2, space="PSUM") as psdp,
        ):
            allp = list(range(PAIRS))
            kts, vgts = {}, {}
            # first pair's inputs lead both queues: the first reduce gates
            # the whole pipeline, so nothing may be enqueued ahead of kt0
            for p in allp:
                kt = ktp.tile([128, NBLK, E], f16, tag="kt")
                vgt = vgp.tile([128, NBLK, E], f16, tag="vg")
                nc.sync.dma_start(out=kt[:], in_=ktw[p])
                nc.scalar.dma_start(out=vgt[:], in_=vg[p])
                kts[p], vgts[p] = kt, vgt
                if p == 0:
                    # consts ride the ACT ring right behind vg0: tri is
                    # needed by the first matmul (~13us), and the kt ring
                    # would deliver it only after all four kt transfers
                    tri = cpool.tile([128, 128], f16)
                    nc.scalar.dma_start(out=tri[:], in_=tri_c[:])
                    ones16 = cpool.tile([16, 128], f16)
                    nc.scalar.dma_start(out=ones16[:], in_=ones_c[:])
                    maskKED = cpool.tile([16, NBLK, E + 1], f16)
                    nc.scalar.dma_start(out=maskKED[:], in_=mke_c[:])

            # front phases, paced pair-by-pair on DVE/ACT; the order edge
            # keeps the scheduler from running later pairs' reduces (gated
            # on late kt DMAs) ahead of this pair's wg in the DVE stream
            from concourse.tile_rust import add_dep_helper
            wgs, gs = {}, {}
            prev_wg = None
            for p in allp:
                # two-stage reduce: DVE adds the e-halves in the packed-16bit
                # 2x mode, the otherwise-idle GPSIMD engine finishes the sum
                t1 = skp.tile([128, NBLK, E // 2], f16, tag="t1")
                red = nc.vector.tensor_tensor(
                    out=t1[:], in0=kts[p][:, :, 0:E // 2],
                    in1=kts[p][:, :, E // 2:E], op=mybir.AluOpType.add,
                )
                sk = skp.tile([128, NBLK], f16, tag="sk")
                nc.vector.tensor_reduce(
                    sk[:], t1[:], mybir.AxisListType.X, mybir.AluOpType.add
                )
                if prev_wg is not None:
                    add_dep_helper(red.ins, prev_wg.ins, sync=False,
                                   reason="reduce after prev pair wg")
                g = gp.tile([128, NBLK], f16, tag="g")
                nc.scalar.activation(
                    g[:], sk[:], mybir.ActivationFunctionType.Exp
                )
                wg = wgp.tile([128, NBLK, E], f16, tag="wg")
                prev_wg = nc.vector.tensor_tensor(
                    out=wg[:], in0=vgts[p][:],
                    in1=g[:].to_broadcast([128, NBLK, E]),
                    op=mybir.AluOpType.mult,
                )
                wgs[p] = wg
                gs[p] = g

            # PSUM stages: 2-pair groups, but with 6 V-bank bufs the next
            # group's matmuls start as soon as one earlier pair drains
            for grp in range(PAIRS // GROUP):
                pairs = list(range(grp * GROUP, (grp + 1) * GROUP))

                pss = {}
                psds = {}
                for p in pairs:
                    # D prefix first: needs only g, runs while wg is built
                    psd = psdp.tile([128, NBLK], f32, tag="psd")
                    psds[p] = psd
                    nc.tensor.matmul(
                        psd[:], lhsT=tri[:], rhs=gs[p][:],
                        start=True, stop=False, skip_group_check=True,
                    )
                    for ci, (k0, k1) in enumerate(CHUNKS):
                        ps = psp.tile([128, k1 - k0, E], f32, tag="ps")
                        nc.tensor.matmul(
                            ps[:], lhsT=tri[:],
                            rhs=wgs[p][:, k0:k1, :],
                            start=True, stop=False, skip_group_check=True,
                        )
                        pss[(p, ci)] = ps

                bsTs = {}
                for p in pairs:
                    # block sums live in row 127; PSUM reads need 32-aligned
                    # partition base: copy rows 96:128, then scatter row 31
                    c32 = c32p.tile([32, NBLK, E + 1], f16, tag="c32")
                    for ci, (k0, k1) in enumerate(CHUNKS):
                        nc.scalar.copy(
                            c32[:, k0:k1, 0:E], pss[(p, ci)][96:128, :, :])
                    nc.scalar.copy(
                        c32[:, :, E:E + 1].rearrange("p k o -> p (k o)"),
                        psds[p][96:128, :])
                    bsT = bsp.tile([NBLK, 1, E + 1], f16, tag="bs")
                    nc.sync.dma_start(out=bsT[:], in_=c32[31:32, :, :],
                                      single_packet=True)
                    bsTs[p] = bsT

                rms = {}
                for p in pairs:
                    rmV = rmp.tile([NBLK, NBLK, E], f16, tag="rmv")
                    nc.vector.tensor_tensor(
                        out=rmV[:], in0=maskKED[:, :, 0:E],
                        in1=bsTs[p][:, :, 0:E].broadcast_to([NBLK, NBLK, E]),
                        op=mybir.AluOpType.mult,
                    )
                    rmD = rmp.tile([NBLK, NBLK], f16, tag="rmd")
                    nc.vector.tensor_tensor(
                        out=rmD[:],
                        in0=maskKED[:, :, E:E + 1].rearrange("a k o -> a (k o)"),
                        in1=bsTs[p][:, :, E:E + 1].rearrange(
                            "a k o -> a (k o)").broadcast_to([NBLK, NBLK]),
                        op=mybir.AluOpType.mult,
                    )
                    rms[p] = (rmV, rmD)

                for p in pairs:
                    for ci, (k0, k1) in enumerate(CHUNKS):
                        nc.tensor.matmul(
                            pss[(p, ci)][:], lhsT=ones16[:],
                            rhs=rms[p][0][:, k0:k1, :],
                            start=False, stop=True, skip_group_check=True,
                        )
                    nc.tensor.matmul(
                        psds[p][:], lhsT=ones16[:], rhs=rms[p][1][:],
                        start=False, stop=True, skip_group_check=True,
                    )

                for p in pairs:
                    r = rp.tile([128, NBLK], f16, tag="r")
                    nc.vector.reciprocal(r[:], psds[p][:])
                    cw = cwp.tile([128, NBLK, E], f16, tag="cw")
                    nc.scalar.copy(cw[:, 0:8, :], pss[(p, 0)][:])
                    nc.scalar.copy(cw[:, 8:16, :], pss[(p, 1)][:])
                    ot = otp.tile([128, NBLK, E], f16, tag="ot")
                    # final normalize: Pool, except the last pair (tail) on
                    # the by-then-idle DVE
                    eng = nc.vector if p == PAIRS - 1 else nc.gpsimd
                    eng.tensor_tensor(
                        out=ot[:], in0=cw[:],
                        in1=r[:].to_broadcast([128, NBLK, E]),
                        op=mybir.AluOpType.mult,
                    )
                    nc.gpsimd.dma_start(out=out[p], in_=ot[:])

    nc.compile()
    return nc


def _get_compiled():
    global _compiled
    if _compiled is None:
        _compiled = _build()
    return _compiled


def _consts():
    f16 = np.float16
    tri = np.triu(np.ones((128, 128), np.float32)).astype(f16)  # tri[c,p]=1 iff c<=p
    ones16 = np.ones((16, 128), f16)
    mk = (np.arange(NBLK)[:, None] < np.arange(NBLK)[None, :]).astype(np.float32)
    mke = np.broadcast_to(mk[:, :, None], (16, NBLK, E + 1)).astype(f16)
    return {
        "tri_c": tri,
        "ones_c": ones16,
        "mke_c": np.ascontiguousarray(mke),
    }


def prep_inputs(keys: np.ndarray, values: np.ndarray, w_score: np.ndarray):
    """Host-side reshard: returns in_maps (list of 8 dicts)."""
    keys = np.asarray(keys, dtype=np.float32)
    values = np.asarray(values, dtype=np.float32)
    w = np.asarray(w_score, dtype=np.float32)

    # [B,S,H,E] -> [B,H,S,E] -> [B*H, NBLK, 128, E] -> [B*H, 128, NBLK, E]
    kt = keys.transpose(0, 2, 1, 3).reshape(B * H, NBLK, 128, E)
    kt = (kt * (-SCALE * w)).transpose(0, 2, 1, 3).astype(np.float16)

    vgf = values.transpose(0, 2, 1, 3).reshape(B * H, NBLK, 128, E)
    vgf = vgf.transpose(0, 2, 1, 3).astype(np.float16)  # [B*H, 128, NBLK, E]

    consts = _consts()
    in_maps = []
    for c in range(NCORES):
        sl = slice(PAIRS * c, PAIRS * (c + 1))
        m = {
            "ktw": np.ascontiguousarray(kt[sl]),
            "vg": np.ascontiguousarray(vgf[sl]),
        }
        m.update(consts)
        in_maps.append(m)
    return in_maps


def assemble_output(results) -> np.ndarray:
    # results[c]["out"]: [PAIRS, 128, NBLK, E]; s = 128*k + partition
    arr = np.stack([np.asarray(r["out"]) for r in results])
    arr = arr.reshape(B * H, 128, NBLK, E).astype(np.float32)
    arr = arr.transpose(0, 2, 1, 3).reshape(B, H, L, E).transpose(0, 2, 1, 3)
    return np.ascontiguousarray(arr)


def kernel(queries=None, keys=None, values=None, w_score=None, b_score=None, attn_mask=None, **_):
    global LAST_RESULTS
    from concourse.bass_utils import run_bass_kernel_spmd

    nc = _get_compiled()
    in_maps = prep_inputs(keys, values, w_score)
    res = run_bass_kernel_spmd(nc, in_maps, core_ids=list(range(NCORES)), trace=TRACE)
    LAST_RESULTS = res
    return assemble_output(res.results)


# revision 44
# speedup vs baseline: 1.0544x; 1.0006x over previous
"""MinusAttention kernel for Trainium2 (8 NeuronCores, Bass/Tile).

Math: score[i,j] = (w.q_i - w.k_j + b) / sqrt(E) with causal mask.
Within a softmax row i the w.q_i and b terms cancel, so

    weights[i,j] = g_j / sum_{j'<=i} g_j',   g_j = exp(-w.k_j / sqrt(E))
    out[i,:]     = (sum_{j<=i} g_j V[j,:]) / (sum_{j<=i} g_j)

i.e. a causal cumulative weighted average of V -- O(S*E) per (b,h) --
and the output does not depend on queries at all.

fp16 end-to-end (PE 1 cycle/col vs 4 for fp32, half the HBM bytes)
with k-major layouts everywhere so every engine/matmul access pattern
is contiguous (measured: transposed ACT writes cost 3.5x, permuted
matmul rhs costs 2x on this HW).  Ragged [7,7,2]-block PSUM chunks (3
per pair, each within a 2KB bank), one merged block-sum scatter per
pair, carry mask multiply in the DVE packed-16bit 2x mode, two-stage
reduce (packed TT + reduce), consts DMA'd from host.  kt+scatter ride
the SP HW ring, vg the ACT HW ring, output stores the gpsimd queue;
the first pair's inputs lead both queues since the first reduce gates
the pipeline.  Final normalize on GPSIMD for the first two pairs (DVE
is busy with front phases then), on DVE for the last two (tail).

Per pair (b,h), s = 128*k + p (p partition, k block 0..15):
  kt  [128,16,64] f16  (host-prescaled by -w/sqrt(E))
  vg  [128,16,65] f16  (col e=64 is ones)
  sk  = reduce_add_e(kt)            DVE 2-stage -> [128,16] f16
  g   = exp(sk)                     ACT
  wg  = vg * g_bcast                DVE
  ps_c = triT @ wg[:,k0:k1,:]       PE f16 (within-block prefix sums)
  c32[:,k0:k1,:] = ps_c[96:128]     ACT (PSUM reads: 32-aligned base)
  bsT = scatter c32 row 31          1 DMA (block sums -> partitions)
  rm  = maskKED * bsT_bcast         DVE 2x (carry terms, k' < k)
  ps_c += ones16 @ rm[:,k0:k1,:]    PE (adds inter-block carries)
  cw[:,k0:k1,:] = ps_c              ACT drains
  r   = 1/cw[:,:,64]                DVE
  ot  = cw[:,:,0:64] * r_bcast      GPSIMD / DVE
  out DMA (f16; host upcasts)
"""

import numpy as np

B, L, S, H, E = 4, 2048, 2048, 8, 64
NCORES = 8
PAIRS = (B * H) // NCORES  # 4 (b,h) pairs per core
NBLK = S // 128  # 16
# PSUM V-chunks: [8,8] blocks x 64 cols -> 2 tiles/pair, each exactly one
# 2KB bank; the denominator column lives in a single shared bank for all
# pairs (its matmul rhs is g itself)
CHUNKS = [(0, 8), (8, 16)]
GROUP = 2
SCALE = np.float32(1.0 / np.sqrt(np.float32(E)))

TRACE = False
LAST_RESULTS = None

_compiled = None


def _build():
    from concourse import bacc
    import concourse.mybir as mybir
    import concourse.tile as tile

    f16 = mybir.dt.float16
    f32 = mybir.dt.float32
    nc = bacc.Bacc("TRN2", target_bir_lowering=False, debug=False)

    ktw = nc.dram_tensor("ktw", [PAIRS, 128, NBLK, E], f16, kind="ExternalInput")
    vg = nc.dram_tensor("vg", [PAIRS, 128, NBLK, E], f16, kind="ExternalInput")
    tri_c = nc.dram_tensor("tri_c", [128, 128], f16, kind="ExternalInput")
    ones_c = nc.dram_tensor("ones_c", [16, 128], f16, kind="ExternalInput")
    # maskKED[k', k, e] = 1 iff k' < k (bcast along e incl. the D col)
    mke_c = nc.dram_tensor("mke_c", [16, NBLK, E + 1], f16, kind="ExternalInput")
    out = nc.dram_tensor("out", [PAIRS, 128, NBLK, E], f16, kind="ExternalOutput")

    with tile.TileContext(nc) as tc:
        with (
            nc.allow_low_precision(reason="fp16 kernel; harness gate is 2e-2"),
            tc.tile_pool(name="const", bufs=1) as cpool,
            tc.tile_pool(name="ktp", bufs=PAIRS) as ktp,
            tc.tile_pool(name="vgp", bufs=PAIRS) as vgp,
            tc.tile_pool(name="skp", bufs=2 * GROUP) as skp,
            tc.tile_pool(name="gp", bufs=2 * GROUP) as gp,
            tc.tile_pool(name="wgp", bufs=PAIRS) as wgp,
            tc.tile_pool(name="c32p", bufs=2 * GROUP) as c32p,
            tc.tile_pool(name="bsp", bufs=2 * GROUP) as bsp,
            tc.tile_pool(name="rmp", bufs=2 * GROUP) as rmp,
            tc.tile_pool(name="rp", bufs=2 * GROUP) as rp,
            tc.tile_pool(name="cwp", bufs=GROUP + 1) as cwp,
            tc.tile_pool(name="otp", bufs=GROUP + 1) as otp,
            tc.tile_pool(name="ps", bufs=6, space="PSUM") as psp,
            tc.tile_pool(name="psd", bufs=2, space="PSUM") as psdp,
        ):
            allp = list(range(PAIRS))
            kts, vgts = {}, {}
            # first pair's inputs lead both queues: the first reduce gates
            # the whole pipeline, so nothing may be enqueued ahead of kt0
            for p in allp:
                kt = ktp.tile([128, NBLK, E], f16, tag="kt")
                vgt = vgp.tile([128, NBLK, E], f16, tag="vg")
                nc.sync.dma_start(out=kt[:], in_=ktw[p])
                nc.scalar.dma_start(out=vgt[:], in_=vg[p])
                kts[p], vgts[p] = kt, vgt
                # consts interleave the kt ring right behind the kt each
                # needs to precede (tri by mm0 ~14us, masks by mask0 ~17us);
                # putting them on the ACT ring would delay exp0 instead
                if p == 0:
                    tri = cpool.tile([128, 128], f16)
                    nc.sync.dma_start(out=tri[:], in_=tri_c[:])
                elif p == 1:
                    ones16 = cpool.tile([16, 128], f16)
                    nc.sync.dma_start(out=ones16[:], in_=ones_c[:])
                elif p == 2:
                    maskKED = cpool.tile([16, NBLK, E + 1], f16)
                    nc.sync.dma_start(out=maskKED[:], in_=mke_c[:])

            # front phases, paced pair-by-pair on DVE/ACT; the order edge
            # keeps the scheduler from running later pairs' reduces (gated
            # on late kt DMAs) ahead of this pair's wg in the DVE stream
            from concourse.tile_rust import add_dep_helper
            wgs, gs = {}, {}
            prev_wg = None
            for p in allp:
                # two-stage reduce: DVE adds the e-halves in the packed-16bit
                # 2x mode, the otherwise-idle GPSIMD engine finishes the sum
                t1 = skp.tile([128, NBLK, E // 2], f16, tag="t1")
                red = nc.vector.tensor_tensor(
                    out=t1[:], in0=kts[p][:, :, 0:E // 2],
                    in1=kts[p][:, :, E // 2:E], op=mybir.AluOpType.add,
                )
                sk = skp.tile([128, NBLK], f16, tag="sk")
                nc.vector.tensor_reduce(
                    sk[:], t1[:], mybir.AxisListType.X, mybir.AluOpType.add
                )
                if prev_wg is not None:
                    add_dep_helper(red.ins, prev_wg.ins, sync=False,
                                   reason="reduce after prev pair wg")
                g = gp.tile([128, NBLK], f16, tag="g")
                nc.scalar.activation(
                    g[:], sk[:], mybir.ActivationFunctionType.Exp
                )
                wg = wgp.tile([128, NBLK, E], f16, tag="wg")
                prev_wg = nc.vector.tensor_tensor(
                    out=wg[:], in0=vgts[p][:],
                    in1=g[:].to_broadcast([128, NBLK, E]),
                    op=mybir.AluOpType.mult,
                )
                wgs[p] = wg
                gs[p] = g

            # PSUM stages: 2-pair groups, but with 6 V-bank bufs the next
            # group's matmuls start as soon as one earlier pair drains
            for grp in range(PAIRS // GROUP):
                pairs = list(range(grp * GROUP, (grp + 1) * GROUP))

                pss = {}
                psds = {}
                for p in pairs:
                    # D prefix first: needs only g, runs while wg is built
                    psd = psdp.tile([128, NBLK], f32, tag="psd")
                    psds[p] = psd
                    nc.tensor.matmul(
                        psd[:], lhsT=tri[:], rhs=gs[p][:],
                        start=True, stop=False, skip_group_check=True,
                    )
                    for ci, (k0, k1) in enumerate(CHUNKS):
                        ps = psp.tile([128, k1 - k0, E], f32, tag="ps")
                        nc.tensor.matmul(
                            ps[:], lhsT=tri[:],
                            rhs=wgs[p][:, k0:k1, :],
                            start=True, stop=False, skip_group_check=True,
                        )
                        pss[(p, ci)] = ps

                bsTs = {}
                for p in pairs:
                    # block sums live in row 127; PSUM reads need 32-aligned
                    # partition base: copy rows 96:128, then scatter row 31
                    c32 = c32p.tile([32, NBLK, E + 1], f16, tag="c32")
                    for ci, (k0, k1) in enumerate(CHUNKS):
                        nc.scalar.copy(
                            c32[:, k0:k1, 0:E], pss[(p, ci)][96:128, :, :])
                    nc.scalar.copy(
                        c32[:, :, E:E + 1].rearrange("p k o -> p (k o)"),
                        psds[p][96:128, :])
                    bsT = bsp.tile([NBLK, 1, E + 1], f16, tag="bs")
                    nc.sync.dma_start(out=bsT[:], in_=c32[31:32, :, :],
                                      single_packet=True)
                    bsTs[p] = bsT

                rms = {}
                for p in pairs:
                    # D mask first: the D chain (carry -> recip -> normalize)
                    # is the longer pole, don't let it slip in the DVE stream
                    rmD = rmp.tile([NBLK, NBLK], f16, tag="rmd")
                    nc.vector.tensor_tensor(
                        out=rmD[:],
                        in0=maskKED[:, :, E:E + 1].rearrange("a k o -> a (k o)"),
                        in1=bsTs[p][:, :, E:E + 1].rearrange(
                            "a k o -> a (k o)").broadcast_to([NBLK, NBLK]),
                        op=mybir.AluOpType.mult,
                    )
                    rmV = rmp.tile([NBLK, NBLK, E], f16, tag="rmv")
                    nc.vector.tensor_tensor(
                        out=rmV[:], in0=maskKED[:, :, 0:E],
                        in1=bsTs[p][:, :, 0:E].broadcast_to([NBLK, NBLK, E]),
                        op=mybir.AluOpType.mult,
                    )
                    rms[p] = (rmV, rmD)

                for p in pairs:
                    for ci, (k0, k1) in enumerate(CHUNKS):
                        nc.tensor.matmul(
                            pss[(p, ci)][:], lhsT=ones16[:],
                            rhs=rms[p][0][:, k0:k1, :],
                            start=False, stop=True, skip_group_check=True,
                        )
                    nc.tensor.matmul(
                        psds[p][:], lhsT=ones16[:], rhs=rms[p][1][:],
                        start=False, stop=True, skip_group_check=True,
                    )

                for p in pairs:
                    r = rp.tile([128, NBLK], f16, tag="r")
                    nc.vector.reciprocal(r[:], psds[p][:])
                    cw = cwp.tile([128, NBLK, E], f16, tag="cw")
                    nc.scalar.copy(cw[:, 0:8, :], pss[(p, 0)][:])
                    nc.scalar.copy(cw[:, 8:16, :], pss[(p, 1)][:])
                    ot = otp.tile([128, NBLK, E], f16, tag="ot")
                    # final normalize: Pool, except the last pair (tail) on
                    # the by-then-idle DVE
                    eng = nc.vector if p == PAIRS - 1 else nc.gpsimd
                    eng.tensor_tensor(
                        out=ot[:], in0=cw[:],
                        in1=r[:].to_broadcast([128, NBLK, E]),
                        op=mybir.AluOpType.mult,
                    )
                    nc.gpsimd.dma_start(out=out[p], in_=ot[:])

    nc.compile()
    return nc


def _get_compiled():
    global _compiled
    if _compiled is None:
        _compiled = _build()
    return _compiled


def _consts():
    f16 = np.float16
    tri = np.triu(np.ones((128, 128), np.float32)).astype(f16)  # tri[c,p]=1 iff c<=p
    ones16 = np.ones((16, 128), f16)
    mk = (np.arange(NBLK)[:, None] < np.arange(NBLK)[None, :]).astype(np.float32)
    mke = np.broadcast_to(mk[:, :, None], (16, NBLK, E + 1)).astype(f16)
    return {
        "tri_c": tri,
        "ones_c": ones16,
        "mke_c": np.ascontiguousarray(mke),
    }


def prep_inputs(keys: np.ndarray, values: np.ndarray, w_score: np.ndarray):
    """Host-side reshard: returns in_maps (list of 8 dicts)."""
    keys = np.asarray(keys, dtype=np.float32)
    values = np.asarray(values, dtype=np.float32)
    w = np.asarray(w_score, dtype=np.float32)

    # [B,S,H,E] -> [B,H,S,E] -> [B*H, NBLK, 128, E] -> [B*H, 128, NBLK, E]
    kt = keys.transpose(0, 2, 1, 3).reshape(B * H, NBLK, 128, E)
    kt = (kt * (-SCALE * w)).transpose(0, 2, 1, 3).astype(np.float16)

    vgf = values.transpose(0, 2, 1, 3).reshape(B * H, NBLK, 128, E)
    vgf = vgf.transpose(0, 2, 1, 3).astype(np.float16)  # [B*H, 128, NBLK, E]

    consts = _consts()
    in_maps = []
    for c in range(NCORES):
        sl = slice(PAIRS * c, PAIRS * (c + 1))
        m = {
            "ktw": np.ascontiguousarray(kt[sl]),
            "vg": np.ascontiguousarray(vgf[sl]),
        }
        m.update(consts)
        in_maps.append(m)
    return in_maps


def assemble_output(results) -> np.ndarray:
    # results[c]["out"]: [PAIRS, 128, NBLK, E]; s = 128*k + partition
    arr = np.stack([np.asarray(r["out"]) for r in results])
    arr = arr.reshape(B * H, 128, NBLK, E).astype(np.float32)
    arr = arr.transpose(0, 2, 1, 3).reshape(B, H, L, E).transpose(0, 2, 1, 3)
    return np.ascontiguousarray(arr)


def kernel(queries=None, keys=None, values=None, w_score=None, b_score=None, attn_mask=None, **_):
    global LAST_RESULTS
    from concourse.bass_utils import run_bass_kernel_spmd

    nc = _get_compiled()
    in_maps = prep_inputs(keys, values, w_score)
    res = run_bass_kernel_spmd(nc, in_maps, core_ids=list(range(NCORES)), trace=TRACE)
    LAST_RESULTS = res
    return assemble_output(res.results)


# revision 45
# speedup vs baseline: 1.0932x; 1.0369x over previous
"""MinusAttention kernel for Trainium2 (8 NeuronCores, Bass/Tile).

Math: score[i,j] = (w.q_i - w.k_j + b) / sqrt(E) with causal mask.
Within a softmax row i the w.q_i and b terms cancel, so

    weights[i,j] = g_j / sum_{j'<=i} g_j',   g_j = exp(-w.k_j / sqrt(E))
    out[i,:]     = (sum_{j<=i} g_j V[j,:]) / (sum_{j<=i} g_j)

i.e. a causal cumulative weighted average of V -- O(S*E) per (b,h) --
and the output does not depend on queries at all.

fp16 end-to-end (PE 1 cycle/col vs 4 for fp32, half the HBM bytes)
with k-major layouts everywhere so every engine/matmul access pattern
is contiguous (measured: transposed ACT writes cost 3.5x, permuted
matmul rhs costs 2x on this HW).  Ragged [7,7,2]-block PSUM chunks (3
per pair, each within a 2KB bank), one merged block-sum scatter per
pair, carry mask multiply in the DVE packed-16bit 2x mode, two-stage
reduce (packed TT + reduce), consts DMA'd from host.  kt+scatter ride
the SP HW ring, vg the ACT HW ring, output stores the gpsimd queue;
the first pair's inputs lead both queues since the first reduce gates
the pipeline.  Final normalize on GPSIMD for the first two pairs (DVE
is busy with front phases then), on DVE for the last two (tail).

Per pair (b,h), s = 128*k + p (p partition, k block 0..15):
  kt  [128,16,64] f16  (host-prescaled by -w/sqrt(E))
  vg  [128,16,65] f16  (col e=64 is ones)
  sk  = reduce_add_e(kt)            DVE 2-stage -> [128,16] f16
  g   = exp(sk)                     ACT
  wg  = vg * g_bcast                DVE
  ps_c = triT @ wg[:,k0:k1,:]       PE f16 (within-block prefix sums)
  c32[:,k0:k1,:] = ps_c[96:128]     ACT (PSUM reads: 32-aligned base)
  bsT = scatter c32 row 31          1 DMA (block sums -> partitions)
  rm  = maskKED * bsT_bcast         DVE 2x (carry terms, k' < k)
  ps_c += ones16 @ rm[:,k0:k1,:]    PE (adds inter-block carries)
  cw[:,k0:k1,:] = ps_c              ACT drains
  r   = 1/cw[:,:,64]                DVE
  ot  = cw[:,:,0:64] * r_bcast      GPSIMD / DVE
  out DMA (f16; host upcasts)
"""

import numpy as np

B, L, S, H, E = 4, 2048, 2048, 8, 64
NCORES = 8
PAIRS = (B * H) // NCORES  # 4 (b,h) pairs per core
NBLK = S // 128  # 16
# PSUM V-chunks: [8,8] blocks x 64 cols -> 2 tiles/pair, each exactly one
# 2KB bank; the denominator column lives in a single shared bank for all
# pairs (its matmul rhs is g itself)
CHUNKS = [(0, 8), (8, 16)]
GROUP = 2
SCALE = np.float32(1.0 / np.sqrt(np.float32(E)))

TRACE = False
LAST_RESULTS = None

_compiled = None


def _build():
    from concourse import bacc
    import concourse.mybir as mybir
    import concourse.tile as tile

    f16 = mybir.dt.float16
    f32 = mybir.dt.float32
    nc = bacc.Bacc("TRN2", target_bir_lowering=False, debug=False)

    ktw = nc.dram_tensor("ktw", [PAIRS, 128, NBLK, E], f16, kind="ExternalInput")
    vg = nc.dram_tensor("vg", [PAIRS, 128, NBLK, E], f16, kind="ExternalInput")
    tri_c = nc.dram_tensor("tri_c", [128, 128], f16, kind="ExternalInput")
    ones_c = nc.dram_tensor("ones_c", [16, 128], f16, kind="ExternalInput")
    # maskKED[k', k, e] = 1 iff k' < k (bcast along e incl. the D col)
    mke_c = nc.dram_tensor("mke_c", [16, NBLK, E + 1], f16, kind="ExternalInput")
    out = nc.dram_tensor("out", [PAIRS, 128, NBLK, E], f16, kind="ExternalOutput")

    with tile.TileContext(nc) as tc:
        with (
            nc.allow_low_precision(reason="fp16 kernel; harness gate is 2e-2"),
            tc.tile_pool(name="const", bufs=1) as cpool,
            tc.tile_pool(name="ktp", bufs=PAIRS) as ktp,
            tc.tile_pool(name="vgp", bufs=PAIRS) as vgp,
            tc.tile_pool(name="skp", bufs=2 * GROUP) as skp,
            tc.tile_pool(name="gp", bufs=2 * GROUP) as gp,
            tc.tile_pool(name="wgp", bufs=PAIRS) as wgp,
            tc.tile_pool(name="c32p", bufs=2 * GROUP) as c32p,
            tc.tile_pool(name="bsp", bufs=2 * GROUP) as bsp,
            tc.tile_pool(name="rmp", bufs=2 * GROUP) as rmp,
            tc.tile_pool(name="rp", bufs=2 * GROUP) as rp,
            tc.tile_pool(name="cwp", bufs=GROUP + 1) as cwp,
            tc.tile_pool(name="otp", bufs=GROUP + 1) as otp,
            tc.tile_pool(name="ps", bufs=6, space="PSUM") as psp,
            tc.tile_pool(name="psd", bufs=2, space="PSUM") as psdp,
        ):
            allp = list(range(PAIRS))
            kts, vgts = {}, {}
            # first pair's inputs lead both queues: the first reduce gates
            # the whole pipeline, so nothing may be enqueued ahead of kt0
            for p in allp:
                kt = ktp.tile([128, NBLK, E], f16, tag="kt")
                vgt = vgp.tile([128, NBLK, E], f16, tag="vg")
                nc.sync.dma_start(out=kt[:], in_=ktw[p])
                nc.scalar.dma_start(out=vgt[:], in_=vg[p])
                kts[p], vgts[p] = kt, vgt
                # consts interleave the kt ring right behind the kt each
                # needs to precede (tri by mm0 ~14us, masks by mask0 ~17us);
                # putting them on the ACT ring would delay exp0 instead
                if p == 0:
                    tri = cpool.tile([128, 128], f16)
                    nc.sync.dma_start(out=tri[:], in_=tri_c[:])
                elif p == 1:
                    ones16 = cpool.tile([16, 128], f16)
                    nc.sync.dma_start(out=ones16[:], in_=ones_c[:])
                elif p == 2:
                    maskKED = cpool.tile([16, NBLK, E + 1], f16)
                    nc.sync.dma_start(out=maskKED[:], in_=mke_c[:])

            # front phases, paced pair-by-pair on DVE/ACT; the order edge
            # keeps the scheduler from running later pairs' reduces (gated
            # on late kt DMAs) ahead of this pair's wg in the DVE stream
            from concourse.tile_rust import add_dep_helper
            wgs, gs = {}, {}
            prev_wg = None
            for p in allp:
                # two-stage reduce: DVE adds the e-halves in the packed-16bit
                # 2x mode, the otherwise-idle GPSIMD engine finishes the sum
                t1 = skp.tile([128, NBLK, E // 2], f16, tag="t1")
                red = nc.vector.tensor_tensor(
                    out=t1[:], in0=kts[p][:, :, 0:E // 2],
                    in1=kts[p][:, :, E // 2:E], op=mybir.AluOpType.add,
                )
                sk = skp.tile([128, NBLK], f16, tag="sk")
                nc.vector.tensor_reduce(
                    sk[:], t1[:], mybir.AxisListType.X, mybir.AluOpType.add
                )
                if prev_wg is not None:
                    add_dep_helper(red.ins, prev_wg.ins, sync=False,
                                   reason="reduce after prev pair wg")
                g = gp.tile([128, NBLK], f16, tag="g")
                nc.scalar.activation(
                    g[:], sk[:], mybir.ActivationFunctionType.Exp
                )
                wg = wgp.tile([128, NBLK, E], f16, tag="wg")
                prev_wg = nc.vector.tensor_tensor(
                    out=wg[:], in0=vgts[p][:],
                    in1=g[:].to_broadcast([128, NBLK, E]),
                    op=mybir.AluOpType.mult,
                )
                wgs[p] = wg
                gs[p] = g

            # PSUM stages: 2-pair groups, but with 6 V-bank bufs the next
            # group's matmuls start as soon as one earlier pair drains
            for grp in range(PAIRS // GROUP):
                pairs = list(range(grp * GROUP, (grp + 1) * GROUP))

                pss = {}
                psds = {}
                for p in pairs:
                    # D prefix first: needs only g, runs while wg is built
                    psd = psdp.tile([128, NBLK], f32, tag="psd")
                    psds[p] = psd
                    nc.tensor.matmul(
                        psd[:], lhsT=tri[:], rhs=gs[p][:],
                        start=True, stop=False, skip_group_check=True,
                    )
                    for ci, (k0, k1) in enumerate(CHUNKS):
                        ps = psp.tile([128, k1 - k0, E], f32, tag="ps")
                        nc.tensor.matmul(
                            ps[:], lhsT=tri[:],
                            rhs=wgs[p][:, k0:k1, :],
                            start=True, stop=False, skip_group_check=True,
                        )
                        pss[(p, ci)] = ps

                bsTs = {}
                for p in pairs:
                    # block sums live in row 127; PSUM reads need 32-aligned
                    # partition base: copy rows 96:128, then scatter row 31
                    c32 = c32p.tile([32, NBLK, E + 1], f16, tag="c32")
                    for ci, (k0, k1) in enumerate(CHUNKS):
                        nc.scalar.copy(
                            c32[:, k0:k1, 0:E], pss[(p, ci)][96:128, :, :])
                    nc.scalar.copy(
                        c32[:, :, E:E + 1].rearrange("p k o -> p (k o)"),
                        psds[p][96:128, :])
                    bsT = bsp.tile([NBLK, 1, E + 1], f16, tag="bs")
                    nc.sync.dma_start(out=bsT[:], in_=c32[31:32, :, :],
                                      single_packet=True)
                    bsTs[p] = bsT

                rms = {}
                for p in pairs:
                    # D mask first: the D chain (carry -> recip -> normalize)
                    # is the longer pole, don't let it slip in the DVE stream
                    rmD = rmp.tile([NBLK, NBLK], f16, tag="rmd")
                    nc.vector.tensor_tensor(
                        out=rmD[:],
                        in0=maskKED[:, :, E:E + 1].rearrange("a k o -> a (k o)"),
                        in1=bsTs[p][:, :, E:E + 1].rearrange(
                            "a k o -> a (k o)").broadcast_to([NBLK, NBLK]),
                        op=mybir.AluOpType.mult,
                    )
                    rmV = rmp.tile([NBLK, NBLK, E], f16, tag="rmv")
                    nc.vector.tensor_tensor(
                        out=rmV[:], in0=maskKED[:, :, 0:E],
                        in1=bsTs[p][:, :, 0:E].broadcast_to([NBLK, NBLK, E]),
                        op=mybir.AluOpType.mult,
                    )
                    rms[p] = (rmV, rmD)

                for p in pairs:
                    for ci, (k0, k1) in enumerate(CHUNKS):
                        nc.tensor.matmul(
                            pss[(p, ci)][:], lhsT=ones16[:],
                            rhs=rms[p][0][:, k0:k1, :],
                            start=False, stop=True, skip_group_check=True,
                        )
                    nc.tensor.matmul(
                        psds[p][:], lhsT=ones16[:], rhs=rms[p][1][:],
                        start=False, stop=True, skip_group_check=True,
                    )

                for p in pairs:
                    r = rp.tile([128, NBLK], f16, tag="r")
                    nc.vector.reciprocal(r[:], psds[p][:])
                    cw = cwp.tile([128, NBLK, E], f16, tag="cw")
                    nc.scalar.copy(cw[:, 0:8, :], pss[(p, 0)][:])
                    nc.scalar.copy(cw[:, 8:16, :], pss[(p, 1)][:])
                    ot = otp.tile([128, NBLK, E], f16, tag="ot")
                    # final normalize on DVE: concurrent GpSimd TTs starve
                    # DVE of SBUF bandwidth (measured 4.5x slowdown), so the
                    # Pool engine only issues the output stores
                    nc.vector.tensor_tensor(
                        out=ot[:], in0=cw[:],
                        in1=r[:].to_broadcast([128, NBLK, E]),
                        op=mybir.AluOpType.mult,
                    )
                    nc.gpsimd.dma_start(out=out[p], in_=ot[:])

    nc.compile()
    return nc


def _get_compiled():
    global _compiled
    if _compiled is None:
        _compiled = _build()
    return _compiled


def _consts():
    f16 = np.float16
    tri = np.triu(np.ones((128, 128), np.float32)).astype(f16)  # tri[c,p]=1 iff c<=p
    ones16 = np.ones((16, 128), f16)
    mk = (np.arange(NBLK)[:, None] < np.arange(NBLK)[None, :]).astype(np.float32)
    mke = np.broadcast_to(mk[:, :, None], (16, NBLK, E + 1)).astype(f16)
    return {
        "tri_c": tri,
        "ones_c": ones16,
        "mke_c": np.ascontiguousarray(mke),
    }


def prep_inputs(keys: np.ndarray, values: np.ndarray, w_score: np.ndarray):
    """Host-side reshard: returns in_maps (list of 8 dicts)."""
    keys = np.asarray(keys, dtype=np.float32)
    values = np.asarray(values, dtype=np.float32)
    w = np.asarray(w_score, dtype=np.float32)

    # [B,S,H,E] -> [B,H,S,E] -> [B*H, NBLK, 128, E] -> [B*H, 128, NBLK, E]
    kt = keys.transpose(0, 2, 1, 3).reshape(B * H, NBLK, 128, E)
    kt = (kt * (-SCALE * w)).transpose(0, 2, 1, 3).astype(np.float16)

    vgf = values.transpose(0, 2, 1, 3).reshape(B * H, NBLK, 128, E)
    vgf = vgf.transpose(0, 2, 1, 3).astype(np.float16)  # [B*H, 128, NBLK, E]

    consts = _consts()
    in_maps = []
    for c in range(NCORES):
        sl = slice(PAIRS * c, PAIRS * (c + 1))
        m = {
            "ktw": np.ascontiguousarray(kt[sl]),
            "vg": np.ascontiguousarray(vgf[sl]),
        }
        m.update(consts)
        in_maps.append(m)
    return in_maps


def assemble_output(results) -> np.ndarray:
    # results[c]["out"]: [PAIRS, 128, NBLK, E]; s = 128*k + partition
    arr = np.stack([np.asarray(r["out"]) for r in results])
    arr = arr.reshape(B * H, 128, NBLK, E).astype(np.float32)
    arr = arr.transpose(0, 2, 1, 3).reshape(B, H, L, E).transpose(0, 2, 1, 3)
    return np.ascontiguousarray(arr)


def kernel(queries=None, keys=None, values=None, w_score=None, b_score=None, attn_mask=None, **_):
    global LAST_RESULTS
    from concourse.bass_utils import run_bass_kernel_spmd

    nc = _get_compiled()
    in_maps = prep_inputs(keys, values, w_score)
    res = run_bass_kernel_spmd(nc, in_maps, core_ids=list(range(NCORES)), trace=TRACE)
    LAST_RESULTS = res
    return assemble_output(res.results)
